# revision 5
# baseline (speedup 1.0000x reference)
"""GCN (4x GCNConv + eval BN + ReLU, global mean pool, 2-layer MLP head) on 8
Trainium2 NeuronCores via Bass/Tile.

Sharding: data-parallel over graphs. 4096 graphs -> 8 cores x 512 contiguous
graphs (batch is sorted). Within a core the 512 graphs form 4 pool groups of
128 graphs; each group's nodes are padded to a multiple of 128 rows so pooling
blocks align with node blocks. Edges live on the core owning their dst node.

Per layer (all on device):
  tt = dinv * (h_local @ W_l)           per-core shard, f16 table
  AllGather tt across the 8 cores       (the only collective)
  agg[v] = dinv[v] * sum_{e: dst=v} tt[src_e]   with self-loops as plain edges
  h = BN_l(relu(agg + b_l))
The segment-sum runs as one-hot matmuls: chunks of 128 dst-sorted edges are
gathered from the tt table by indirect DMA, lhsT = is_equal(dst_local, iota),
and the PE accumulates chunks into PSUM per 128-node dst block.

The symmetric GCN normalization dinv[src]*dinv[dst] is folded: dinv[src] into
the table, dinv[dst] into the block epilogue; the self-loop term t*1/deg is
exactly a self-edge under this folding.

All data-dependent structure is precomputed host-side into per-core meta
arrays; the device program is identical across cores (SPMD).
"""

import os
import numpy as np

import concourse.bass as bass
import concourse.tile as tile
from concourse import mybir, bacc, bass_utils
from concourse.masks import make_identity

P = 128
H = 128
N_CORES = 8
N_GRAPHS = 4096
GPC = N_GRAPHS // N_CORES      # graphs per core
GB = 4                         # pool groups (of 128 graphs) per core
BN_EPS = 1e-5
UNROLL = 4                     # chunk-loop unroll / buffer sets per half

F32 = mybir.dt.float32
F16 = mybir.dt.float16
I32 = mybir.dt.int32

LAST_EXEC_NS = None
_CACHE = {}


def _preprocess(x, src, dst, batch, dinv):
    """Host-side sharding: node remap + per-core padded meta arrays."""
    N = x.shape[0]
    graph_start = np.searchsorted(batch, np.arange(N_GRAPHS + 1))
    seg_rows = np.zeros((N_CORES, GB), dtype=np.int64)
    for c in range(N_CORES):
        for g in range(GB):
            g0 = c * GPC + g * P
            seg_rows[c, g] = graph_start[g0 + P] - graph_start[g0]
    C2 = int(np.ceil(seg_rows.max() / P))     # node blocks per pool group
    NBLK = GB * C2                            # node blocks per core
    NPC = NBLK * P                            # padded nodes per core

    newid = np.zeros(N, dtype=np.int64)
    for c in range(N_CORES):
        for g in range(GB):
            g0 = c * GPC + g * P
            r0, r1 = graph_start[g0], graph_start[g0 + P]
            newid[r0:r1] = c * NPC + g * C2 * P + np.arange(r1 - r0)

    x_loc = np.zeros((N_CORES, NPC, H), dtype=np.float32)
    dinvb = np.ones((N_CORES, P, NBLK), dtype=np.float32)
    glocb = np.full((N_CORES, P, NBLK), -1.0, dtype=np.float32)
    invcnt = np.ones((N_CORES, P, GB), dtype=np.float32)
    loc_all = newid % NPC
    core_all = newid // NPC
    for c in range(N_CORES):
        m = core_all == c
        loc = loc_all[m]
        x_loc[c, loc] = x[m]
        dinvb[c, loc % P, loc // P] = dinv[m]
        gl = (batch[m] - c * GPC).astype(np.int64)      # 0..GPC-1
        glocb[c, loc % P, loc // P] = (gl % P).astype(np.float32)
        cnt = np.zeros(GPC, dtype=np.float64)
        np.add.at(cnt, gl, 1.0)
        invcnt[c] = (1.0 / np.maximum(cnt, 1.0)).reshape(GB, P).T.astype(np.float32)

    # edges + self-loops grouped by dst block
    e_src_g = newid[src]
    e_dst_core = core_all[dst]
    e_dst_loc = loc_all[dst]
    counts = np.zeros((N_CORES, NBLK), dtype=np.int64)
    np.add.at(counts, (e_dst_core, e_dst_loc // P), 1)
    np.add.at(counts, (core_all, loc_all // P), 1)     # self-loops
    C_max = int(np.ceil(counts.max() / P))
    NCH = NBLK * C_max + (UNROLL - 1)   # slack for loop bounds analysis

    srcg = np.zeros((N_CORES, P, NCH), dtype=np.int32)
    dstl = np.full((N_CORES, P, NCH), -1.0, dtype=np.float32)
    for c in range(N_CORES):
        m = e_dst_core == c
        es = np.concatenate([e_src_g[m], newid[core_all == c]])
        ed = np.concatenate([e_dst_loc[m], loc_all[core_all == c]])
        order = np.argsort(ed // P, kind="stable")
        es, ed = es[order], ed[order]
        blk = ed // P
        blk_starts = np.searchsorted(blk, np.arange(NBLK))
        slot = np.arange(len(es)) - blk_starts[blk]
        col = blk * C_max + slot // P
        row = slot % P
        srcg[c, row, col] = es.astype(np.int32)
        dstl[c, row, col] = (ed % P).astype(np.float32)

    return dict(C2=C2, NBLK=NBLK, NPC=NPC, C_max=C_max, NCH=NCH,
                x_loc=x_loc, dinvb=dinvb, glocb=glocb, invcnt=invcnt,
                srcg=srcg, dstl=dstl)


def _build(C2, NBLK, NPC, C_max, NCH, hb2_val):
    table_dt = F16
    nc = bacc.Bacc("TRN2", target_bir_lowering=False, debug=False,
                   num_devices=N_CORES)
    x_d = nc.dram_tensor("x_loc", [NPC, H], F32, kind="ExternalInput")
    srcg_d = nc.dram_tensor("srcg", [P, NCH], I32, kind="ExternalInput")
    dstl_d = nc.dram_tensor("dstl", [P, NCH], table_dt, kind="ExternalInput")
    dinvb_d = nc.dram_tensor("dinvb", [P, NBLK], F32, kind="ExternalInput")
    glocb_d = nc.dram_tensor("glocb", [P, NBLK], F32, kind="ExternalInput")
    invcnt_d = nc.dram_tensor("invcnt", [P, GB], F32, kind="ExternalInput")
    W_d = nc.dram_tensor("Wsb", [H, 4 * H], F32, kind="ExternalInput")
    brep_d = nc.dram_tensor("brep", [P, 4 * H], F32, kind="ExternalInput")
    srep_d = nc.dram_tensor("srep", [P, 4 * H], F32, kind="ExternalInput")
    b2rep_d = nc.dram_tensor("b2rep", [P, 4 * H], F32, kind="ExternalInput")
    iota16_d = nc.dram_tensor("iota16", [P, P], table_dt, kind="ExternalInput")
    iota32_d = nc.dram_tensor("iota32", [P, P], F32, kind="ExternalInput")
    hW1_d = nc.dram_tensor("hW1", [H, H], F32, kind="ExternalInput")
    hb1rep_d = nc.dram_tensor("hb1rep", [P, H], F32, kind="ExternalInput")
    hW2_d = nc.dram_tensor("hW2", [H, 1], F32, kind="ExternalInput")
    out_d = nc.dram_tensor("out", [GPC, 1], F32, kind="ExternalOutput")

    t_loc = [nc.dram_tensor(f"t_loc{l}", [NPC, H], table_dt) for l in range(4)]
    T_full = [nc.dram_tensor(f"T_full{l}", [N_CORES * NPC, H], table_dt)
              for l in range(4)]

    n_loop = ((C_max - 1) // UNROLL) * UNROLL
    n_tail = (C_max - 1) - n_loop

    with tile.TileContext(nc) as tc:
        with (
            tc.tile_pool(name="persist", bufs=1) as pp,
            tc.tile_pool(name="stagea", bufs=3) as sap,
            tc.tile_pool(name="pool2", bufs=2) as wp2,
            tc.tile_pool(name="psum_agg", bufs=1, space="PSUM") as psagg_tp,
            tc.tile_pool(name="psum_a", bufs=1, space="PSUM") as psa_tp,
            tc.tile_pool(name="psum_p", bufs=1, space="PSUM") as psp_tp,
        ):
            h_sb = pp.tile([P, NBLK * H], F32)
            t_sb = pp.tile([P, NBLK * H], table_dt)
            srcg = pp.tile([P, NCH], I32)
            dstl = pp.tile([P, NCH], table_dt)
            dinvb = pp.tile([P, NBLK], F32)
            glocb = pp.tile([P, NBLK], F32)
            invcnt = pp.tile([P, GB], F32)
            W_sb = pp.tile([H, 4 * H], F32)
            brep = pp.tile([P, 4 * H], F32)
            srep = pp.tile([P, 4 * H], F32)
            b2rep = pp.tile([P, 4 * H], F32)
            iota16 = pp.tile([P, P], table_dt)
            iota32 = pp.tile([P, P], F32)
            hW1_sb = pp.tile([H, H], F32)
            hb1rep = pp.tile([P, H], F32)
            hW2_sb = pp.tile([H, 1], F32)
            ident = pp.tile([P, P], F32)
            z2all = pp.tile([1, GPC], F32)
            for sb, d in [(srcg, srcg_d), (dstl, dstl_d), (dinvb, dinvb_d),
                          (glocb, glocb_d), (invcnt, invcnt_d), (W_sb, W_d),
                          (brep, brep_d), (srep, srep_d), (b2rep, b2rep_d),
                          (iota16, iota16_d), (iota32, iota32_d),
                          (hW1_sb, hW1_d), (hb1rep, hb1rep_d), (hW2_sb, hW2_d)]:
                nc.sync.dma_start(sb[:], d[:])
            make_identity(nc, ident[:])
            for b in range(NBLK):
                nc.sync.dma_start(h_sb[:, b * H:(b + 1) * H],
                                  x_d[b * P:(b + 1) * P, :])

            # chunk buffer sets: [half][k]
            sets = [[(pp.tile([P, H], table_dt, name=f"g{h}_{k}"),
                      pp.tile([P, P], table_dt, name=f"oh{h}_{k}"),
                      pp.tile([P, 1], I32, name=f"idx{h}_{k}"))
                     for k in range(UNROLL)] for h in range(2)]
            ps_half = [psagg_tp.tile([P, H], F32, space="PSUM", name=f"psagg{h}")
                       for h in range(2)]
            dv_st = [pp.tile([P, 1], F32, name=f"dv{h}") for h in range(2)]
            ep = [[pp.tile([P, H], F32, name=f"ep{h}_{i}") for i in range(2)]
                  for h in range(2)]

            def agg_chunk(col, sk, ps, start, T_l):
                g, oh, idx = sk
                nc.vector.tensor_copy(idx[:], srcg[:, bass.ds(col, 1)])
                nc.gpsimd.indirect_dma_start(
                    out=g[:], out_offset=None, in_=T_l[:],
                    in_offset=bass.IndirectOffsetOnAxis(ap=idx[:], axis=0))
                nc.vector.tensor_tensor(
                    out=oh[:], in0=dstl[:, bass.ds(col, 1)].to_broadcast([P, P]),
                    in1=iota16[:], op=mybir.AluOpType.is_equal)
                nc.tensor.matmul(ps[:], lhsT=oh[:], rhs=g[:],
                                 start=start, stop=False, skip_group_check=True)

            for l in range(4):
                ls = slice(l * H, (l + 1) * H)
                with nc.named_scope(f"stageA{l}"):
                    for b in range(NBLK):
                        trp = psa_tp.tile([P, H], F32, space="PSUM",
                                          name="trp")
                        nc.tensor.transpose(out=trp[:],
                                            in_=h_sb[:, b * H:(b + 1) * H],
                                            identity=ident[:])
                        hT = sap.tile([P, H], F32, name="hT")
                        nc.scalar.copy(hT[:], trp[:])
                        tps = psa_tp.tile([P, H], F32, space="PSUM",
                                          name="tps")
                        nc.tensor.matmul(tps[:], lhsT=hT[:], rhs=W_sb[:, ls],
                                         start=True, stop=True,
                                         skip_group_check=True)
                        nc.scalar.activation(t_sb[:, b * H:(b + 1) * H], tps[:],
                                             mybir.ActivationFunctionType.Copy,
                                             scale=dinvb[:, b:b + 1])
                        nc.sync.dma_start(t_loc[l][b * P:(b + 1) * P, :],
                                          t_sb[:, b * H:(b + 1) * H])

                with nc.named_scope(f"ag{l}"):
                    nc.gpsimd.collective_compute(
                        "AllGather", mybir.AluOpType.bypass,
                        replica_groups=[list(range(N_CORES))],
                        ins=[t_loc[l][:]], outs=[T_full[l][:]])

                with nc.named_scope(f"agg{l}"):
                    with tc.For_i(0, NBLK, 2) as b2:
                        for half in range(2):
                            base = (b2 + half) * C_max
                            ps = ps_half[half]
                            agg_chunk(base, sets[half][0], ps, True, T_full[l])
                            if n_loop > 0:
                                with tc.For_i(base + 1, base + 1 + n_loop,
                                              UNROLL) as j:
                                    for k in range(UNROLL):
                                        agg_chunk(j + k, sets[half][k], ps,
                                                  False, T_full[l])
                            for i in range(n_tail):
                                agg_chunk(base + 1 + n_loop + i,
                                          sets[half][(i + 1) % UNROLL], ps,
                                          False, T_full[l])
                            # epilogue: h = BN(relu(dinv*psum + b))
                            hcol = bass.ds((b2 + half) * H, H)
                            e0, e1 = ep[half]
                            nc.vector.tensor_copy(dv_st[half][:],
                                                  dinvb[:, bass.ds(b2 + half, 1)])
                            nc.vector.tensor_scalar(
                                e0[:], ps[:], dv_st[half][:], None,
                                mybir.AluOpType.mult)
                            nc.vector.tensor_tensor(
                                out=e1[:], in0=e0[:], in1=brep[:, ls],
                                op=mybir.AluOpType.add)
                            nc.scalar.activation(
                                e0[:], e1[:], mybir.ActivationFunctionType.Relu)
                            nc.vector.tensor_tensor(
                                out=e1[:], in0=e0[:], in1=srep[:, ls],
                                op=mybir.AluOpType.mult)
                            nc.vector.tensor_tensor(
                                out=h_sb[:, hcol], in0=e1[:], in1=b2rep[:, ls],
                                op=mybir.AluOpType.add)

            # ---- global mean pool + head
            with nc.named_scope("pool"):
                for gb in range(GB):
                    pps = psp_tp.tile([P, H], F32, space="PSUM", name="pps")
                    for k in range(C2):
                        b = gb * C2 + k
                        oh32 = wp2.tile([P, P], F32, name="oh32")
                        nc.vector.tensor_tensor(
                            out=oh32[:],
                            in0=glocb[:, b:b + 1].to_broadcast([P, P]),
                            in1=iota32[:], op=mybir.AluOpType.is_equal)
                        nc.tensor.matmul(pps[:], lhsT=oh32[:],
                                         rhs=h_sb[:, b * H:(b + 1) * H],
                                         start=(k == 0), stop=(k == C2 - 1),
                                         skip_group_check=True)
                    pooled = wp2.tile([P, H], F32, name="pooled")
                    nc.vector.tensor_scalar(pooled[:], pps[:],
                                            invcnt[:, gb:gb + 1], None,
                                            mybir.AluOpType.mult)
                    # head: relu(pooled @ hW1 + hb1) @ hW2 + hb2
                    trp = psp_tp.tile([P, H], F32, space="PSUM", name="htr")
                    nc.tensor.transpose(out=trp[:], in_=pooled[:],
                                        identity=ident[:])
                    poolT = wp2.tile([P, H], F32, name="poolT")
                    nc.scalar.copy(poolT[:], trp[:])
                    z1ps = psp_tp.tile([P, H], F32, space="PSUM", name="z1ps")
                    nc.tensor.matmul(z1ps[:], lhsT=poolT[:], rhs=hW1_sb[:],
                                     start=True, stop=True,
                                     skip_group_check=True)
                    r1 = wp2.tile([P, H], F32, name="r1")
                    nc.vector.tensor_tensor(out=r1[:], in0=z1ps[:],
                                            in1=hb1rep[:],
                                            op=mybir.AluOpType.add)
                    nc.scalar.activation(r1[:], r1[:],
                                         mybir.ActivationFunctionType.Relu)
                    tr2 = psp_tp.tile([P, H], F32, space="PSUM", name="htr")
                    nc.tensor.transpose(out=tr2[:], in_=r1[:], identity=ident[:])
                    r1T = wp2.tile([P, H], F32, name="r1T")
                    nc.scalar.copy(r1T[:], tr2[:])
                    z2ps = psp_tp.tile([1, P], F32, space="PSUM", name="z2ps")
                    nc.tensor.matmul(z2ps[:], lhsT=hW2_sb[:], rhs=r1T[:],
                                     start=True, stop=True,
                                     skip_group_check=True)
                    nc.vector.tensor_scalar(
                        z2all[0:1, gb * P:(gb + 1) * P], z2ps[:],
                        float(hb2_val), None, mybir.AluOpType.add)
                nc.sync.dma_start(out_d[:, 0:1], z2all[0:1, :])

    nc.compile()
    return nc


def kernel(**inputs):
    global LAST_EXEC_NS
    x = np.ascontiguousarray(np.asarray(inputs["x"], dtype=np.float32))
    ei = np.asarray(inputs["edge_index"]).astype(np.int64)
    batch = np.asarray(inputs["batch"]).astype(np.int64)
    Ws = np.asarray(inputs["Ws"], dtype=np.float32)
    bs = np.asarray(inputs["bs"], dtype=np.float32)
    gammas = np.asarray(inputs["gammas"], dtype=np.float32)
    betas = np.asarray(inputs["betas"], dtype=np.float32)
    bn_means = np.asarray(inputs["bn_means"], dtype=np.float32)
    bn_vars = np.asarray(inputs["bn_vars"], dtype=np.float32)
    hW1 = np.asarray(inputs["hW1"], dtype=np.float32)
    hb1 = np.asarray(inputs["hb1"], dtype=np.float32)
    hW2 = np.asarray(inputs["hW2"], dtype=np.float32)
    hb2 = np.asarray(inputs["hb2"], dtype=np.float32)

    src, dst = ei[0], ei[1]
    N = x.shape[0]
    deg = np.bincount(dst, minlength=N).astype(np.float64) + 1.0
    dinv = (1.0 / np.sqrt(deg)).astype(np.float32)

    meta = _preprocess(x, src, dst, batch, dinv)
    C2, NBLK, NPC, C_max, NCH = (meta[k] for k in
                                 ("C2", "NBLK", "NPC", "C_max", "NCH"))

    key = (C2, NBLK, NPC, C_max, NCH, float(hb2[0]))
    if key not in _CACHE:
        _CACHE[key] = _build(C2, NBLK, NPC, C_max, NCH, float(hb2[0]))
    nc = _CACHE[key]

    # replicated constant arrays
    s_l = gammas / np.sqrt(bn_vars + BN_EPS)            # [4, H]
    b2_l = betas - bn_means * s_l                        # [4, H]
    Wsb = np.ascontiguousarray(Ws.transpose(1, 0, 2).reshape(H, 4 * H))
    brep = np.broadcast_to(bs.reshape(1, 4 * H), (P, 4 * H)).copy()
    srep = np.broadcast_to(s_l.reshape(1, 4 * H), (P, 4 * H)).copy()
    b2rep = np.broadcast_to(b2_l.reshape(1, 4 * H), (P, 4 * H)).copy()
    iota16 = np.broadcast_to(np.arange(P, dtype=np.float16)[None, :],
                             (P, P)).copy()
    iota32 = iota16.astype(np.float32)
    hb1rep = np.broadcast_to(hb1[None, :], (P, H)).copy()

    in_maps = []
    for c in range(N_CORES):
        in_maps.append({
            "x_loc": meta["x_loc"][c],
            "srcg": meta["srcg"][c],
            "dstl": meta["dstl"][c].astype(np.float16),
            "dinvb": meta["dinvb"][c],
            "glocb": meta["glocb"][c],
            "invcnt": meta["invcnt"][c],
            "Wsb": Wsb, "brep": brep, "srep": srep, "b2rep": b2rep,
            "iota16": iota16, "iota32": iota32,
            "hW1": hW1, "hb1rep": hb1rep, "hW2": hW2,
        })

    trace = os.environ.get("BASS_GCN_TRACE", "") == "1"
    if trace:
        bass_utils.upload_artifacts = lambda tmpdir: "local://" + tmpdir
    res = bass_utils.run_bass_kernel_spmd(nc, in_maps, list(range(N_CORES)),
                                          trace=trace)
    LAST_EXEC_NS = res.exec_time_ns
    if res.exec_time_ns is not None:
        print(f"HW exec time: {res.exec_time_ns} ns")

    out = np.concatenate([res.results[c]["out"] for c in range(N_CORES)],
                         axis=0).astype(np.float32)
    return out


# revision 6
# speedup vs baseline: 1.3232x; 1.3232x over previous
"""GCN (4x GCNConv + eval BN + ReLU, global mean pool, 2-layer MLP head) on 8
Trainium2 NeuronCores via Bass/Tile.

Sharding: data-parallel over graphs. 4096 graphs -> 8 cores x 512 contiguous
graphs (batch is sorted). Within a core the 512 graphs form 4 pool groups of
128 graphs; each group's nodes are padded to a multiple of 128 rows so pooling
blocks align with node blocks. Edges live on the core owning their dst node.

Per layer (all on device):
  tt = dinv * (h_local @ W_l)           per-core shard, f16 table
  AllGather tt across the 8 cores       (the only collective)
  agg[v] = dinv[v] * sum_{e: dst=v} tt[src_e]   with self-loops as plain edges
  h = BN_l(relu(agg + b_l))
The segment-sum runs as one-hot matmuls: chunks of 128 dst-sorted edges are
gathered from the tt table by indirect DMA, lhsT = is_equal(dst_local, iota),
and the PE accumulates chunks into PSUM per 128-node dst block.

The symmetric GCN normalization dinv[src]*dinv[dst] is folded: dinv[src] into
the table, dinv[dst] into the block epilogue; the self-loop term t*1/deg is
exactly a self-edge under this folding.

All data-dependent structure is precomputed host-side into per-core meta
arrays; the device program is identical across cores (SPMD).
"""

import os
import numpy as np

import concourse.bass as bass
import concourse.tile as tile
from concourse import mybir, bacc, bass_utils
from concourse.masks import make_identity

P = 128
H = 128
N_CORES = 8
N_GRAPHS = 4096
GPC = N_GRAPHS // N_CORES      # graphs per core
GB = 4                         # pool groups (of 128 graphs) per core
BN_EPS = 1e-5
UNROLL = 8                     # chunk-loop unroll / buffer sets per half

F32 = mybir.dt.float32
F16 = mybir.dt.float16
I32 = mybir.dt.int32

LAST_EXEC_NS = None
_CACHE = {}


def _preprocess(x, src, dst, batch, dinv):
    """Host-side sharding: node remap + per-core padded meta arrays."""
    N = x.shape[0]
    graph_start = np.searchsorted(batch, np.arange(N_GRAPHS + 1))
    seg_rows = np.zeros((N_CORES, GB), dtype=np.int64)
    for c in range(N_CORES):
        for g in range(GB):
            g0 = c * GPC + g * P
            seg_rows[c, g] = graph_start[g0 + P] - graph_start[g0]
    C2 = int(np.ceil(seg_rows.max() / P))     # node blocks per pool group
    NBLK = GB * C2                            # node blocks per core
    NPC = NBLK * P                            # padded nodes per core

    newid = np.zeros(N, dtype=np.int64)
    for c in range(N_CORES):
        for g in range(GB):
            g0 = c * GPC + g * P
            r0, r1 = graph_start[g0], graph_start[g0 + P]
            newid[r0:r1] = c * NPC + g * C2 * P + np.arange(r1 - r0)

    x_loc = np.zeros((N_CORES, NPC, H), dtype=np.float32)
    dinvb = np.ones((N_CORES, P, NBLK), dtype=np.float32)
    glocb = np.full((N_CORES, P, NBLK), -1.0, dtype=np.float32)
    invcnt = np.ones((N_CORES, P, GB), dtype=np.float32)
    loc_all = newid % NPC
    core_all = newid // NPC
    for c in range(N_CORES):
        m = core_all == c
        loc = loc_all[m]
        x_loc[c, loc] = x[m]
        dinvb[c, loc % P, loc // P] = dinv[m]
        gl = (batch[m] - c * GPC).astype(np.int64)      # 0..GPC-1
        glocb[c, loc % P, loc // P] = (gl % P).astype(np.float32)
        cnt = np.zeros(GPC, dtype=np.float64)
        np.add.at(cnt, gl, 1.0)
        invcnt[c] = (1.0 / np.maximum(cnt, 1.0)).reshape(GB, P).T.astype(np.float32)

    # edges + self-loops grouped by dst block
    e_src_g = newid[src]
    e_dst_core = core_all[dst]
    e_dst_loc = loc_all[dst]
    counts = np.zeros((N_CORES, NBLK), dtype=np.int64)
    np.add.at(counts, (e_dst_core, e_dst_loc // P), 1)
    np.add.at(counts, (core_all, loc_all // P), 1)     # self-loops
    C_max = int(np.ceil(counts.max() / P))
    NCH = NBLK * C_max + (UNROLL - 1)   # slack for loop bounds analysis

    srcg = np.zeros((N_CORES, P, NCH), dtype=np.int32)
    dstl = np.full((N_CORES, P, NCH), -1.0, dtype=np.float32)
    for c in range(N_CORES):
        m = e_dst_core == c
        es = np.concatenate([e_src_g[m], newid[core_all == c]])
        ed = np.concatenate([e_dst_loc[m], loc_all[core_all == c]])
        order = np.argsort(ed // P, kind="stable")
        es, ed = es[order], ed[order]
        blk = ed // P
        blk_starts = np.searchsorted(blk, np.arange(NBLK))
        slot = np.arange(len(es)) - blk_starts[blk]
        col = blk * C_max + slot // P
        row = slot % P
        srcg[c, row, col] = es.astype(np.int32)
        dstl[c, row, col] = (ed % P).astype(np.float32)

    return dict(C2=C2, NBLK=NBLK, NPC=NPC, C_max=C_max, NCH=NCH,
                x_loc=x_loc, dinvb=dinvb, glocb=glocb, invcnt=invcnt,
                srcg=srcg, dstl=dstl)


def _build(C2, NBLK, NPC, C_max, NCH, hb2_val):
    table_dt = F16
    nc = bacc.Bacc("TRN2", target_bir_lowering=False, debug=False,
                   num_devices=N_CORES)
    x_d = nc.dram_tensor("x_loc", [NPC, H], F32, kind="ExternalInput")
    srcg_d = nc.dram_tensor("srcg", [P, NCH], I32, kind="ExternalInput")
    dstl_d = nc.dram_tensor("dstl", [P, NCH], table_dt, kind="ExternalInput")
    dinvb_d = nc.dram_tensor("dinvb", [P, NBLK], F32, kind="ExternalInput")
    glocb_d = nc.dram_tensor("glocb", [P, NBLK], F32, kind="ExternalInput")
    invcnt_d = nc.dram_tensor("invcnt", [P, GB], F32, kind="ExternalInput")
    W_d = nc.dram_tensor("Wsb", [H, 4 * H], F32, kind="ExternalInput")
    brep_d = nc.dram_tensor("brep", [P, 4 * H], F32, kind="ExternalInput")
    srep_d = nc.dram_tensor("srep", [P, 4 * H], F32, kind="ExternalInput")
    b2rep_d = nc.dram_tensor("b2rep", [P, 4 * H], F32, kind="ExternalInput")
    iota16_d = nc.dram_tensor("iota16", [P, P], table_dt, kind="ExternalInput")
    iota32_d = nc.dram_tensor("iota32", [P, P], F32, kind="ExternalInput")
    hW1_d = nc.dram_tensor("hW1", [H, H], F32, kind="ExternalInput")
    hb1rep_d = nc.dram_tensor("hb1rep", [P, H], F32, kind="ExternalInput")
    hW2_d = nc.dram_tensor("hW2", [H, 1], F32, kind="ExternalInput")
    out_d = nc.dram_tensor("out", [GPC, 1], F32, kind="ExternalOutput")

    t_loc = [nc.dram_tensor(f"t_loc{l}", [NPC, H], table_dt) for l in range(4)]
    T_full = [nc.dram_tensor(f"T_full{l}", [N_CORES * NPC, H], table_dt)
              for l in range(4)]

    n_loop = ((C_max - 1) // UNROLL) * UNROLL
    n_tail = (C_max - 1) - n_loop

    with tile.TileContext(nc) as tc:
        with (
            tc.tile_pool(name="persist", bufs=1) as pp,
            tc.tile_pool(name="stagea", bufs=3) as sap,
            tc.tile_pool(name="pool2", bufs=2) as wp2,
            tc.tile_pool(name="psum_agg", bufs=1, space="PSUM") as psagg_tp,
            tc.tile_pool(name="psum_a", bufs=1, space="PSUM") as psa_tp,
            tc.tile_pool(name="psum_p", bufs=1, space="PSUM") as psp_tp,
        ):
            h_sb = pp.tile([P, NBLK * H], F32)
            t_sb = pp.tile([P, NBLK * H], table_dt)
            srcg = pp.tile([P, NCH], I32)
            dstl = pp.tile([P, NCH], table_dt)
            dinvb = pp.tile([P, NBLK], F32)
            glocb = pp.tile([P, NBLK], F32)
            invcnt = pp.tile([P, GB], F32)
            W_sb = pp.tile([H, 4 * H], F32)
            brep = pp.tile([P, 4 * H], F32)
            srep = pp.tile([P, 4 * H], F32)
            b2rep = pp.tile([P, 4 * H], F32)
            iota16 = pp.tile([P, P], table_dt)
            iota32 = pp.tile([P, P], F32)
            hW1_sb = pp.tile([H, H], F32)
            hb1rep = pp.tile([P, H], F32)
            hW2_sb = pp.tile([H, 1], F32)
            ident = pp.tile([P, P], F32)
            z2all = pp.tile([1, GPC], F32)
            for sb, d in [(srcg, srcg_d), (dstl, dstl_d), (dinvb, dinvb_d),
                          (glocb, glocb_d), (invcnt, invcnt_d), (W_sb, W_d),
                          (brep, brep_d), (srep, srep_d), (b2rep, b2rep_d),
                          (iota16, iota16_d), (iota32, iota32_d),
                          (hW1_sb, hW1_d), (hb1rep, hb1rep_d), (hW2_sb, hW2_d)]:
                nc.sync.dma_start(sb[:], d[:])
            make_identity(nc, ident[:])
            for b in range(NBLK):
                nc.sync.dma_start(h_sb[:, b * H:(b + 1) * H],
                                  x_d[b * P:(b + 1) * P, :])

            # chunk buffer sets: [half][k]
            sets = [[(pp.tile([P, H], table_dt, name=f"g{h}_{k}"),
                      pp.tile([P, P], table_dt, name=f"oh{h}_{k}"),
                      pp.tile([P, 1], I32, name=f"idx{h}_{k}"))
                     for k in range(UNROLL)] for h in range(2)]
            ps_half = [psagg_tp.tile([P, H], F32, space="PSUM", name=f"psagg{h}")
                       for h in range(2)]
            dv_st = [pp.tile([P, 1], F32, name=f"dv{h}") for h in range(2)]
            ep = [[pp.tile([P, H], F32, name=f"ep{h}_{i}") for i in range(2)]
                  for h in range(2)]

            def agg_chunk(col, sk, ps, start, T_l):
                g, oh, idx = sk
                nc.vector.tensor_copy(idx[:], srcg[:, bass.ds(col, 1)])
                nc.gpsimd.indirect_dma_start(
                    out=g[:], out_offset=None, in_=T_l[:],
                    in_offset=bass.IndirectOffsetOnAxis(ap=idx[:], axis=0))
                nc.vector.tensor_tensor(
                    out=oh[:], in0=dstl[:, bass.ds(col, 1)].to_broadcast([P, P]),
                    in1=iota16[:], op=mybir.AluOpType.is_equal)
                nc.tensor.matmul(ps[:], lhsT=oh[:], rhs=g[:],
                                 start=start, stop=False, skip_group_check=True)

            for l in range(4):
                ls = slice(l * H, (l + 1) * H)
                with nc.named_scope(f"stageA{l}"):
                    for b in range(NBLK):
                        trp = psa_tp.tile([P, H], F32, space="PSUM",
                                          name="trp")
                        nc.tensor.transpose(out=trp[:],
                                            in_=h_sb[:, b * H:(b + 1) * H],
                                            identity=ident[:])
                        hT = sap.tile([P, H], F32, name="hT")
                        nc.scalar.copy(hT[:], trp[:])
                        tps = psa_tp.tile([P, H], F32, space="PSUM",
                                          name="tps")
                        nc.tensor.matmul(tps[:], lhsT=hT[:], rhs=W_sb[:, ls],
                                         start=True, stop=True,
                                         skip_group_check=True)
                        nc.scalar.activation(t_sb[:, b * H:(b + 1) * H], tps[:],
                                             mybir.ActivationFunctionType.Copy,
                                             scale=dinvb[:, b:b + 1])
                        nc.sync.dma_start(t_loc[l][b * P:(b + 1) * P, :],
                                          t_sb[:, b * H:(b + 1) * H])

                with nc.named_scope(f"ag{l}"):
                    nc.gpsimd.collective_compute(
                        "AllGather", mybir.AluOpType.bypass,
                        replica_groups=[list(range(N_CORES))],
                        ins=[t_loc[l][:]], outs=[T_full[l][:]])

                with nc.named_scope(f"agg{l}"):
                    with tc.For_i(0, NBLK, 2) as b2:
                        bases = [b2 * C_max, (b2 + 1) * C_max]
                        for half in range(2):
                            agg_chunk(bases[half], sets[half][0],
                                      ps_half[half], True, T_full[l])
                        if n_loop > 0:
                            with tc.For_i(1, 1 + n_loop, UNROLL) as j:
                                for k in range(UNROLL):
                                    for half in range(2):
                                        agg_chunk(bases[half] + j + k,
                                                  sets[half][k], ps_half[half],
                                                  False, T_full[l])
                        for i in range(n_tail):
                            for half in range(2):
                                agg_chunk(bases[half] + 1 + n_loop + i,
                                          sets[half][(i + 1) % UNROLL],
                                          ps_half[half], False, T_full[l])
                        for half in range(2):
                            # epilogue: h = BN(relu(dinv*psum + b))
                            ps = ps_half[half]
                            hcol = bass.ds((b2 + half) * H, H)
                            e0, e1 = ep[half]
                            nc.vector.tensor_copy(dv_st[half][:],
                                                  dinvb[:, bass.ds(b2 + half, 1)])
                            nc.vector.tensor_scalar(
                                e0[:], ps[:], dv_st[half][:], None,
                                mybir.AluOpType.mult)
                            nc.vector.tensor_tensor(
                                out=e1[:], in0=e0[:], in1=brep[:, ls],
                                op=mybir.AluOpType.add)
                            nc.scalar.activation(
                                e0[:], e1[:], mybir.ActivationFunctionType.Relu)
                            nc.vector.tensor_tensor(
                                out=e1[:], in0=e0[:], in1=srep[:, ls],
                                op=mybir.AluOpType.mult)
                            nc.vector.tensor_tensor(
                                out=h_sb[:, hcol], in0=e1[:], in1=b2rep[:, ls],
                                op=mybir.AluOpType.add)

            # ---- global mean pool + head
            with nc.named_scope("pool"):
                for gb in range(GB):
                    pps = psp_tp.tile([P, H], F32, space="PSUM", name="pps")
                    for k in range(C2):
                        b = gb * C2 + k
                        oh32 = wp2.tile([P, P], F32, name="oh32")
                        nc.vector.tensor_tensor(
                            out=oh32[:],
                            in0=glocb[:, b:b + 1].to_broadcast([P, P]),
                            in1=iota32[:], op=mybir.AluOpType.is_equal)
                        nc.tensor.matmul(pps[:], lhsT=oh32[:],
                                         rhs=h_sb[:, b * H:(b + 1) * H],
                                         start=(k == 0), stop=(k == C2 - 1),
                                         skip_group_check=True)
                    pooled = wp2.tile([P, H], F32, name="pooled")
                    nc.vector.tensor_scalar(pooled[:], pps[:],
                                            invcnt[:, gb:gb + 1], None,
                                            mybir.AluOpType.mult)
                    # head: relu(pooled @ hW1 + hb1) @ hW2 + hb2
                    trp = psp_tp.tile([P, H], F32, space="PSUM", name="htr")
                    nc.tensor.transpose(out=trp[:], in_=pooled[:],
                                        identity=ident[:])
                    poolT = wp2.tile([P, H], F32, name="poolT")
                    nc.scalar.copy(poolT[:], trp[:])
                    z1ps = psp_tp.tile([P, H], F32, space="PSUM", name="z1ps")
                    nc.tensor.matmul(z1ps[:], lhsT=poolT[:], rhs=hW1_sb[:],
                                     start=True, stop=True,
                                     skip_group_check=True)
                    r1 = wp2.tile([P, H], F32, name="r1")
                    nc.vector.tensor_tensor(out=r1[:], in0=z1ps[:],
                                            in1=hb1rep[:],
                                            op=mybir.AluOpType.add)
                    nc.scalar.activation(r1[:], r1[:],
                                         mybir.ActivationFunctionType.Relu)
                    tr2 = psp_tp.tile([P, H], F32, space="PSUM", name="htr")
                    nc.tensor.transpose(out=tr2[:], in_=r1[:], identity=ident[:])
                    r1T = wp2.tile([P, H], F32, name="r1T")
                    nc.scalar.copy(r1T[:], tr2[:])
                    z2ps = psp_tp.tile([1, P], F32, space="PSUM", name="z2ps")
                    nc.tensor.matmul(z2ps[:], lhsT=hW2_sb[:], rhs=r1T[:],
                                     start=True, stop=True,
                                     skip_group_check=True)
                    nc.vector.tensor_scalar(
                        z2all[0:1, gb * P:(gb + 1) * P], z2ps[:],
                        float(hb2_val), None, mybir.AluOpType.add)
                nc.sync.dma_start(out_d[:, 0:1], z2all[0:1, :])

    nc.compile()
    return nc


def kernel(**inputs):
    global LAST_EXEC_NS
    x = np.ascontiguousarray(np.asarray(inputs["x"], dtype=np.float32))
    ei = np.asarray(inputs["edge_index"]).astype(np.int64)
    batch = np.asarray(inputs["batch"]).astype(np.int64)
    Ws = np.asarray(inputs["Ws"], dtype=np.float32)
    bs = np.asarray(inputs["bs"], dtype=np.float32)
    gammas = np.asarray(inputs["gammas"], dtype=np.float32)
    betas = np.asarray(inputs["betas"], dtype=np.float32)
    bn_means = np.asarray(inputs["bn_means"], dtype=np.float32)
    bn_vars = np.asarray(inputs["bn_vars"], dtype=np.float32)
    hW1 = np.asarray(inputs["hW1"], dtype=np.float32)
    hb1 = np.asarray(inputs["hb1"], dtype=np.float32)
    hW2 = np.asarray(inputs["hW2"], dtype=np.float32)
    hb2 = np.asarray(inputs["hb2"], dtype=np.float32)

    src, dst = ei[0], ei[1]
    N = x.shape[0]
    deg = np.bincount(dst, minlength=N).astype(np.float64) + 1.0
    dinv = (1.0 / np.sqrt(deg)).astype(np.float32)

    meta = _preprocess(x, src, dst, batch, dinv)
    C2, NBLK, NPC, C_max, NCH = (meta[k] for k in
                                 ("C2", "NBLK", "NPC", "C_max", "NCH"))

    key = (C2, NBLK, NPC, C_max, NCH, float(hb2[0]))
    if key not in _CACHE:
        _CACHE[key] = _build(C2, NBLK, NPC, C_max, NCH, float(hb2[0]))
    nc = _CACHE[key]

    # replicated constant arrays
    s_l = gammas / np.sqrt(bn_vars + BN_EPS)            # [4, H]
    b2_l = betas - bn_means * s_l                        # [4, H]
    Wsb = np.ascontiguousarray(Ws.transpose(1, 0, 2).reshape(H, 4 * H))
    brep = np.broadcast_to(bs.reshape(1, 4 * H), (P, 4 * H)).copy()
    srep = np.broadcast_to(s_l.reshape(1, 4 * H), (P, 4 * H)).copy()
    b2rep = np.broadcast_to(b2_l.reshape(1, 4 * H), (P, 4 * H)).copy()
    iota16 = np.broadcast_to(np.arange(P, dtype=np.float16)[None, :],
                             (P, P)).copy()
    iota32 = iota16.astype(np.float32)
    hb1rep = np.broadcast_to(hb1[None, :], (P, H)).copy()

    in_maps = []
    for c in range(N_CORES):
        in_maps.append({
            "x_loc": meta["x_loc"][c],
            "srcg": meta["srcg"][c],
            "dstl": meta["dstl"][c].astype(np.float16),
            "dinvb": meta["dinvb"][c],
            "glocb": meta["glocb"][c],
            "invcnt": meta["invcnt"][c],
            "Wsb": Wsb, "brep": brep, "srep": srep, "b2rep": b2rep,
            "iota16": iota16, "iota32": iota32,
            "hW1": hW1, "hb1rep": hb1rep, "hW2": hW2,
        })

    trace = os.environ.get("BASS_GCN_TRACE", "") == "1"
    if trace:
        bass_utils.upload_artifacts = lambda tmpdir: "local://" + tmpdir
    res = bass_utils.run_bass_kernel_spmd(nc, in_maps, list(range(N_CORES)),
                                          trace=trace)
    LAST_EXEC_NS = res.exec_time_ns
    if res.exec_time_ns is not None:
        print(f"HW exec time: {res.exec_time_ns} ns")

    out = np.concatenate([res.results[c]["out"] for c in range(N_CORES)],
                         axis=0).astype(np.float32)
    return out


# revision 7
# speedup vs baseline: 1.7877x; 1.3510x over previous
"""GCN (4x GCNConv + eval BN + ReLU, global mean pool, 2-layer MLP head) on 8
Trainium2 NeuronCores via Bass/Tile.

Sharding: data-parallel over graphs. 4096 graphs -> 8 cores x 512 contiguous
graphs (batch is sorted). Within a core the 512 graphs form 4 pool groups of
128 graphs; each group's nodes are padded to a multiple of 128 rows so pooling
blocks align with node blocks. Edges live on the core owning their dst node.

Per layer (all on device):
  tt = dinv * (h_local @ W_l)           per-core shard, f16 table
  AllGather tt across the 8 cores       (the only collective)
  agg[v] = dinv[v] * sum_{e: dst=v} tt[src_e]   with self-loops as plain edges
  h = BN_l(relu(agg + b_l))
The segment-sum runs as one-hot matmuls: chunks of 128 dst-sorted edges are
gathered from the tt table by indirect DMA, lhsT = is_equal(dst_local, iota),
and the PE accumulates chunks into PSUM per 128-node dst block.

The symmetric GCN normalization dinv[src]*dinv[dst] is folded: dinv[src] into
the table, dinv[dst] into the block epilogue; the self-loop term t*1/deg is
exactly a self-edge under this folding.

All data-dependent structure is precomputed host-side into per-core meta
arrays; the device program is identical across cores (SPMD).
"""

import os
import numpy as np

import concourse.bass as bass
import concourse.tile as tile
from concourse import mybir, bacc, bass_utils
from concourse.masks import make_identity

P = 128
H = 128
N_CORES = 8
N_GRAPHS = 4096
GPC = N_GRAPHS // N_CORES      # graphs per core
GB = 4                         # pool groups (of 128 graphs) per core
BN_EPS = 1e-5
UNROLL = 8                     # chunk-loop unroll / buffer sets per half

F32 = mybir.dt.float32
F16 = mybir.dt.float16
I32 = mybir.dt.int32

LAST_EXEC_NS = None
_CACHE = {}


def _preprocess(x, src, dst, batch, dinv):
    """Host-side sharding: node remap + per-core padded meta arrays."""
    N = x.shape[0]
    graph_start = np.searchsorted(batch, np.arange(N_GRAPHS + 1))
    seg_rows = np.zeros((N_CORES, GB), dtype=np.int64)
    for c in range(N_CORES):
        for g in range(GB):
            g0 = c * GPC + g * P
            seg_rows[c, g] = graph_start[g0 + P] - graph_start[g0]
    C2 = int(np.ceil(seg_rows.max() / P))     # node blocks per pool group
    NBLK = GB * C2                            # node blocks per core
    NPC = NBLK * P                            # padded nodes per core

    newid = np.zeros(N, dtype=np.int64)
    for c in range(N_CORES):
        for g in range(GB):
            g0 = c * GPC + g * P
            r0, r1 = graph_start[g0], graph_start[g0 + P]
            newid[r0:r1] = c * NPC + g * C2 * P + np.arange(r1 - r0)

    x_loc = np.zeros((N_CORES, NPC, H), dtype=np.float32)
    dinvb = np.ones((N_CORES, P, NBLK), dtype=np.float32)
    glocb = np.full((N_CORES, P, NBLK), -1.0, dtype=np.float32)
    invcnt = np.ones((N_CORES, P, GB), dtype=np.float32)
    loc_all = newid % NPC
    core_all = newid // NPC
    for c in range(N_CORES):
        m = core_all == c
        loc = loc_all[m]
        x_loc[c, loc] = x[m]
        dinvb[c, loc % P, loc // P] = dinv[m]
        gl = (batch[m] - c * GPC).astype(np.int64)      # 0..GPC-1
        glocb[c, loc % P, loc // P] = (gl % P).astype(np.float32)
        cnt = np.zeros(GPC, dtype=np.float64)
        np.add.at(cnt, gl, 1.0)
        invcnt[c] = (1.0 / np.maximum(cnt, 1.0)).reshape(GB, P).T.astype(np.float32)

    # edges + self-loops grouped by dst block
    e_src_g = newid[src]
    e_dst_core = core_all[dst]
    e_dst_loc = loc_all[dst]
    counts = np.zeros((N_CORES, NBLK), dtype=np.int64)
    np.add.at(counts, (e_dst_core, e_dst_loc // P), 1)
    np.add.at(counts, (core_all, loc_all // P), 1)     # self-loops
    C_max = int(np.ceil(counts.max() / P))
    NCH = NBLK * C_max + (UNROLL - 1)   # slack for loop bounds analysis

    srcg = np.zeros((N_CORES, P, NCH), dtype=np.int32)
    dstl = np.full((N_CORES, P, NCH), -1.0, dtype=np.float32)
    for c in range(N_CORES):
        m = e_dst_core == c
        es = np.concatenate([e_src_g[m], newid[core_all == c]])
        ed = np.concatenate([e_dst_loc[m], loc_all[core_all == c]])
        order = np.argsort(ed // P, kind="stable")
        es, ed = es[order], ed[order]
        blk = ed // P
        blk_starts = np.searchsorted(blk, np.arange(NBLK))
        slot = np.arange(len(es)) - blk_starts[blk]
        col = blk * C_max + slot // P
        row = slot % P
        srcg[c, row, col] = es.astype(np.int32)
        dstl[c, row, col] = (ed % P).astype(np.float32)

    return dict(C2=C2, NBLK=NBLK, NPC=NPC, C_max=C_max, NCH=NCH,
                x_loc=x_loc, dinvb=dinvb, glocb=glocb, invcnt=invcnt,
                srcg=srcg, dstl=dstl)


def _build(C2, NBLK, NPC, C_max, NCH, hb2_val):
    table_dt = F16
    nc = bacc.Bacc("TRN2", target_bir_lowering=False, debug=False,
                   num_devices=N_CORES)
    x_d = nc.dram_tensor("x_loc", [NPC, H], F32, kind="ExternalInput")
    srcg_d = nc.dram_tensor("srcg", [P, NCH], I32, kind="ExternalInput")
    dstl_d = nc.dram_tensor("dstl", [P, NCH], table_dt, kind="ExternalInput")
    dinvb_d = nc.dram_tensor("dinvb", [P, NBLK], F32, kind="ExternalInput")
    glocb_d = nc.dram_tensor("glocb", [P, NBLK], F32, kind="ExternalInput")
    invcnt_d = nc.dram_tensor("invcnt", [P, GB], F32, kind="ExternalInput")
    W_d = nc.dram_tensor("Wsb", [H, 4 * H], F32, kind="ExternalInput")
    brep_d = nc.dram_tensor("brep", [P, 4 * H], F32, kind="ExternalInput")
    srep_d = nc.dram_tensor("srep", [P, 4 * H], F32, kind="ExternalInput")
    b2rep_d = nc.dram_tensor("b2rep", [P, 4 * H], F32, kind="ExternalInput")
    iota16_d = nc.dram_tensor("iota16", [P, P], table_dt, kind="ExternalInput")
    iota32_d = nc.dram_tensor("iota32", [P, P], F32, kind="ExternalInput")
    hW1_d = nc.dram_tensor("hW1", [H, H], F32, kind="ExternalInput")
    hb1rep_d = nc.dram_tensor("hb1rep", [P, H], F32, kind="ExternalInput")
    hW2_d = nc.dram_tensor("hW2", [H, 1], F32, kind="ExternalInput")
    out_d = nc.dram_tensor("out", [GPC, 1], F32, kind="ExternalOutput")

    t_loc = [nc.dram_tensor(f"t_loc{l}", [NPC, H], table_dt) for l in range(4)]
    T_full = [nc.dram_tensor(f"T_full{l}", [N_CORES * NPC, H], table_dt)
              for l in range(4)]

    n_loop = ((C_max - 1) // UNROLL) * UNROLL
    n_tail = (C_max - 1) - n_loop

    with tile.TileContext(nc) as tc:
        with (
            tc.tile_pool(name="persist", bufs=1) as pp,
            tc.tile_pool(name="stagea", bufs=3) as sap,
            tc.tile_pool(name="pool2", bufs=2) as wp2,
            tc.tile_pool(name="psum_agg", bufs=1, space="PSUM") as psagg_tp,
            tc.tile_pool(name="psum_a", bufs=1, space="PSUM") as psa_tp,
            tc.tile_pool(name="psum_p", bufs=1, space="PSUM") as psp_tp,
        ):
            h_sb = pp.tile([P, NBLK * H], F32)
            t_sb = pp.tile([P, NBLK * H], table_dt)
            srcg = pp.tile([P, NCH], I32)
            dstl = pp.tile([P, NCH], table_dt)
            dinvb = pp.tile([P, NBLK], F32)
            glocb = pp.tile([P, NBLK], F32)
            invcnt = pp.tile([P, GB], F32)
            W_sb = pp.tile([H, 4 * H], F32)
            brep = pp.tile([P, 4 * H], F32)
            srep = pp.tile([P, 4 * H], F32)
            b2rep = pp.tile([P, 4 * H], F32)
            iota16 = pp.tile([P, P], table_dt)
            iota32 = pp.tile([P, P], F32)
            hW1_sb = pp.tile([H, H], F32)
            hb1rep = pp.tile([P, H], F32)
            hW2_sb = pp.tile([H, 1], F32)
            ident = pp.tile([P, P], F32)
            z2all = pp.tile([1, GPC], F32)
            for sb, d in [(srcg, srcg_d), (dstl, dstl_d), (dinvb, dinvb_d),
                          (glocb, glocb_d), (invcnt, invcnt_d), (W_sb, W_d),
                          (brep, brep_d), (srep, srep_d), (b2rep, b2rep_d),
                          (iota16, iota16_d), (iota32, iota32_d),
                          (hW1_sb, hW1_d), (hb1rep, hb1rep_d), (hW2_sb, hW2_d)]:
                nc.sync.dma_start(sb[:], d[:])
            make_identity(nc, ident[:])
            for b in range(NBLK):
                nc.sync.dma_start(h_sb[:, b * H:(b + 1) * H],
                                  x_d[b * P:(b + 1) * P, :])

            # chunk buffer sets: [half][k]
            sets = [[(pp.tile([P, H], table_dt, name=f"g{h}_{k}"),
                      pp.tile([P, P], table_dt, name=f"oh{h}_{k}"))
                     for k in range(UNROLL)] for h in range(2)]
            ps_half = [psagg_tp.tile([P, H], F32, space="PSUM", name=f"psagg{h}")
                       for h in range(2)]
            def agg_chunk(col, sk, ps, start, stop, T_l):
                g, oh = sk
                nc.gpsimd.indirect_dma_start(
                    out=g[:], out_offset=None, in_=T_l[:],
                    in_offset=bass.IndirectOffsetOnAxis(ap=srcg[:, col:col + 1],
                                                        axis=0))
                nc.vector.tensor_tensor(
                    out=oh[:], in0=dstl[:, col:col + 1].to_broadcast([P, P]),
                    in1=iota16[:], op=mybir.AluOpType.is_equal)
                nc.tensor.matmul(ps[:], lhsT=oh[:], rhs=g[:],
                                 start=start, stop=stop, skip_group_check=True)

            for l in range(4):
                ls = slice(l * H, (l + 1) * H)
                with nc.named_scope(f"stageA{l}"):
                    for b in range(NBLK):
                        trp = psa_tp.tile([P, H], F32, space="PSUM",
                                          name="trp")
                        nc.tensor.transpose(out=trp[:],
                                            in_=h_sb[:, b * H:(b + 1) * H],
                                            identity=ident[:])
                        hT = sap.tile([P, H], F32, name="hT")
                        nc.scalar.copy(hT[:], trp[:])
                        tps = psa_tp.tile([P, H], F32, space="PSUM",
                                          name="tps")
                        nc.tensor.matmul(tps[:], lhsT=hT[:], rhs=W_sb[:, ls],
                                         start=True, stop=True,
                                         skip_group_check=True)
                        nc.scalar.activation(t_sb[:, b * H:(b + 1) * H], tps[:],
                                             mybir.ActivationFunctionType.Copy,
                                             scale=dinvb[:, b:b + 1])
                        nc.sync.dma_start(t_loc[l][b * P:(b + 1) * P, :],
                                          t_sb[:, b * H:(b + 1) * H])

                with nc.named_scope(f"ag{l}"):
                    nc.gpsimd.collective_compute(
                        "AllGather", mybir.AluOpType.bypass,
                        replica_groups=[list(range(N_CORES))],
                        ins=[t_loc[l][:]], outs=[T_full[l][:]])

                with nc.named_scope(f"agg{l}"):
                    for bp in range(NBLK // 2):
                        blocks = [2 * bp, 2 * bp + 1]
                        for j in range(C_max):
                            for half in range(2):
                                agg_chunk(blocks[half] * C_max + j,
                                          sets[half][j % UNROLL],
                                          ps_half[half], j == 0,
                                          j == C_max - 1, T_full[l])
                        for half in range(2):
                            # epilogue: h = BN(relu(dinv*psum + b))
                            b = blocks[half]
                            ps = ps_half[half]
                            e0 = wp2.tile([P, H], F32, name=f"e0_{half}")
                            e1 = wp2.tile([P, H], F32, name=f"e1_{half}")
                            nc.vector.tensor_scalar(
                                e0[:], ps[:], dinvb[:, b:b + 1], None,
                                mybir.AluOpType.mult)
                            nc.vector.tensor_tensor(
                                out=e1[:], in0=e0[:], in1=brep[:, ls],
                                op=mybir.AluOpType.add)
                            nc.scalar.activation(
                                e0[:], e1[:], mybir.ActivationFunctionType.Relu)
                            nc.vector.tensor_tensor(
                                out=e1[:], in0=e0[:], in1=srep[:, ls],
                                op=mybir.AluOpType.mult)
                            nc.vector.tensor_tensor(
                                out=h_sb[:, b * H:(b + 1) * H], in0=e1[:],
                                in1=b2rep[:, ls],
                                op=mybir.AluOpType.add)

            # ---- global mean pool + head
            with nc.named_scope("pool"):
                for gb in range(GB):
                    pps = psp_tp.tile([P, H], F32, space="PSUM", name="pps")
                    for k in range(C2):
                        b = gb * C2 + k
                        oh32 = wp2.tile([P, P], F32, name="oh32")
                        nc.vector.tensor_tensor(
                            out=oh32[:],
                            in0=glocb[:, b:b + 1].to_broadcast([P, P]),
                            in1=iota32[:], op=mybir.AluOpType.is_equal)
                        nc.tensor.matmul(pps[:], lhsT=oh32[:],
                                         rhs=h_sb[:, b * H:(b + 1) * H],
                                         start=(k == 0), stop=(k == C2 - 1),
                                         skip_group_check=True)
                    pooled = wp2.tile([P, H], F32, name="pooled")
                    nc.vector.tensor_scalar(pooled[:], pps[:],
                                            invcnt[:, gb:gb + 1], None,
                                            mybir.AluOpType.mult)
                    # head: relu(pooled @ hW1 + hb1) @ hW2 + hb2
                    trp = psp_tp.tile([P, H], F32, space="PSUM", name="htr")
                    nc.tensor.transpose(out=trp[:], in_=pooled[:],
                                        identity=ident[:])
                    poolT = wp2.tile([P, H], F32, name="poolT")
                    nc.scalar.copy(poolT[:], trp[:])
                    z1ps = psp_tp.tile([P, H], F32, space="PSUM", name="z1ps")
                    nc.tensor.matmul(z1ps[:], lhsT=poolT[:], rhs=hW1_sb[:],
                                     start=True, stop=True,
                                     skip_group_check=True)
                    r1 = wp2.tile([P, H], F32, name="r1")
                    nc.vector.tensor_tensor(out=r1[:], in0=z1ps[:],
                                            in1=hb1rep[:],
                                            op=mybir.AluOpType.add)
                    nc.scalar.activation(r1[:], r1[:],
                                         mybir.ActivationFunctionType.Relu)
                    tr2 = psp_tp.tile([P, H], F32, space="PSUM", name="htr")
                    nc.tensor.transpose(out=tr2[:], in_=r1[:], identity=ident[:])
                    r1T = wp2.tile([P, H], F32, name="r1T")
                    nc.scalar.copy(r1T[:], tr2[:])
                    z2ps = psp_tp.tile([1, P], F32, space="PSUM", name="z2ps")
                    nc.tensor.matmul(z2ps[:], lhsT=hW2_sb[:], rhs=r1T[:],
                                     start=True, stop=True,
                                     skip_group_check=True)
                    nc.vector.tensor_scalar(
                        z2all[0:1, gb * P:(gb + 1) * P], z2ps[:],
                        float(hb2_val), None, mybir.AluOpType.add)
                nc.sync.dma_start(out_d[:, 0:1], z2all[0:1, :])

    nc.compile()
    return nc


def kernel(**inputs):
    global LAST_EXEC_NS
    x = np.ascontiguousarray(np.asarray(inputs["x"], dtype=np.float32))
    ei = np.asarray(inputs["edge_index"]).astype(np.int64)
    batch = np.asarray(inputs["batch"]).astype(np.int64)
    Ws = np.asarray(inputs["Ws"], dtype=np.float32)
    bs = np.asarray(inputs["bs"], dtype=np.float32)
    gammas = np.asarray(inputs["gammas"], dtype=np.float32)
    betas = np.asarray(inputs["betas"], dtype=np.float32)
    bn_means = np.asarray(inputs["bn_means"], dtype=np.float32)
    bn_vars = np.asarray(inputs["bn_vars"], dtype=np.float32)
    hW1 = np.asarray(inputs["hW1"], dtype=np.float32)
    hb1 = np.asarray(inputs["hb1"], dtype=np.float32)
    hW2 = np.asarray(inputs["hW2"], dtype=np.float32)
    hb2 = np.asarray(inputs["hb2"], dtype=np.float32)

    src, dst = ei[0], ei[1]
    N = x.shape[0]
    deg = np.bincount(dst, minlength=N).astype(np.float64) + 1.0
    dinv = (1.0 / np.sqrt(deg)).astype(np.float32)

    meta = _preprocess(x, src, dst, batch, dinv)
    C2, NBLK, NPC, C_max, NCH = (meta[k] for k in
                                 ("C2", "NBLK", "NPC", "C_max", "NCH"))

    key = (C2, NBLK, NPC, C_max, NCH, float(hb2[0]))
    if key not in _CACHE:
        _CACHE[key] = _build(C2, NBLK, NPC, C_max, NCH, float(hb2[0]))
    nc = _CACHE[key]

    # replicated constant arrays
    s_l = gammas / np.sqrt(bn_vars + BN_EPS)            # [4, H]
    b2_l = betas - bn_means * s_l                        # [4, H]
    Wsb = np.ascontiguousarray(Ws.transpose(1, 0, 2).reshape(H, 4 * H))
    brep = np.broadcast_to(bs.reshape(1, 4 * H), (P, 4 * H)).copy()
    srep = np.broadcast_to(s_l.reshape(1, 4 * H), (P, 4 * H)).copy()
    b2rep = np.broadcast_to(b2_l.reshape(1, 4 * H), (P, 4 * H)).copy()
    iota16 = np.broadcast_to(np.arange(P, dtype=np.float16)[None, :],
                             (P, P)).copy()
    iota32 = iota16.astype(np.float32)
    hb1rep = np.broadcast_to(hb1[None, :], (P, H)).copy()

    in_maps = []
    for c in range(N_CORES):
        in_maps.append({
            "x_loc": meta["x_loc"][c],
            "srcg": meta["srcg"][c],
            "dstl": meta["dstl"][c].astype(np.float16),
            "dinvb": meta["dinvb"][c],
            "glocb": meta["glocb"][c],
            "invcnt": meta["invcnt"][c],
            "Wsb": Wsb, "brep": brep, "srep": srep, "b2rep": b2rep,
            "iota16": iota16, "iota32": iota32,
            "hW1": hW1, "hb1rep": hb1rep, "hW2": hW2,
        })

    trace = os.environ.get("BASS_GCN_TRACE", "") == "1"
    if trace:
        bass_utils.upload_artifacts = lambda tmpdir: "local://" + tmpdir
    res = bass_utils.run_bass_kernel_spmd(nc, in_maps, list(range(N_CORES)),
                                          trace=trace)
    LAST_EXEC_NS = res.exec_time_ns
    if res.exec_time_ns is not None:
        print(f"HW exec time: {res.exec_time_ns} ns")

    out = np.concatenate([res.results[c]["out"] for c in range(N_CORES)],
                         axis=0).astype(np.float32)
    return out


# revision 8
# speedup vs baseline: 2.0562x; 1.1502x over previous
"""GCN (4x GCNConv + eval BN + ReLU, global mean pool, 2-layer MLP head) on 8
Trainium2 NeuronCores via Bass/Tile.

Sharding: data-parallel over graphs. 4096 graphs -> 8 cores x 512 contiguous
graphs (batch is sorted). Within a core the 512 graphs form 4 pool groups of
128 graphs; each group's nodes are padded to a multiple of 128 rows so pooling
blocks align with node blocks. Edges live on the core owning their dst node.

Per layer (all on device):
  tt = dinv * (h_local @ W_l)           per-core shard, f16 table
  AllGather tt across the 8 cores       (the only collective)
  agg[v] = dinv[v] * sum_{e: dst=v} tt[src_e]   with self-loops as plain edges
  h = BN_l(relu(agg + b_l))
The segment-sum runs as one-hot matmuls: chunks of 128 dst-sorted edges are
gathered from the tt table by indirect DMA, lhsT = is_equal(dst_local, iota),
and the PE accumulates chunks into PSUM per 128-node dst block.

The symmetric GCN normalization dinv[src]*dinv[dst] is folded: dinv[src] into
the table, dinv[dst] into the block epilogue; the self-loop term t*1/deg is
exactly a self-edge under this folding.

All data-dependent structure is precomputed host-side into per-core meta
arrays; the device program is identical across cores (SPMD).
"""

import os
import numpy as np

import concourse.bass as bass
import concourse.tile as tile
from concourse import mybir, bacc, bass_utils
from concourse.masks import make_identity

P = 128
H = 128
N_CORES = 8
N_GRAPHS = 4096
GPC = N_GRAPHS // N_CORES      # graphs per core
GB = 4                         # pool groups (of 128 graphs) per core
BN_EPS = 1e-5
UNROLL = 8                     # chunk-loop unroll / buffer sets per half

F32 = mybir.dt.float32
F16 = mybir.dt.float16
I32 = mybir.dt.int32

LAST_EXEC_NS = None
_CACHE = {}


def _preprocess(x, src, dst, batch, dinv):
    """Host-side sharding: node remap + per-core padded meta arrays."""
    N = x.shape[0]
    graph_start = np.searchsorted(batch, np.arange(N_GRAPHS + 1))
    seg_rows = np.zeros((N_CORES, GB), dtype=np.int64)
    for c in range(N_CORES):
        for g in range(GB):
            g0 = c * GPC + g * P
            seg_rows[c, g] = graph_start[g0 + P] - graph_start[g0]
    C2 = int(np.ceil(seg_rows.max() / P))     # node blocks per pool group
    NBLK = GB * C2                            # node blocks per core
    NPC = NBLK * P                            # padded nodes per core

    newid = np.zeros(N, dtype=np.int64)
    for c in range(N_CORES):
        for g in range(GB):
            g0 = c * GPC + g * P
            r0, r1 = graph_start[g0], graph_start[g0 + P]
            newid[r0:r1] = c * NPC + g * C2 * P + np.arange(r1 - r0)

    x_loc = np.zeros((N_CORES, NPC, H), dtype=np.float32)
    dinvb = np.ones((N_CORES, P, NBLK), dtype=np.float32)
    glocb = np.full((N_CORES, P, NBLK), -1.0, dtype=np.float32)
    invcnt = np.ones((N_CORES, P, GB), dtype=np.float32)
    loc_all = newid % NPC
    core_all = newid // NPC
    for c in range(N_CORES):
        m = core_all == c
        loc = loc_all[m]
        x_loc[c, loc] = x[m]
        dinvb[c, loc % P, loc // P] = dinv[m]
        gl = (batch[m] - c * GPC).astype(np.int64)      # 0..GPC-1
        glocb[c, loc % P, loc // P] = (gl % P).astype(np.float32)
        cnt = np.zeros(GPC, dtype=np.float64)
        np.add.at(cnt, gl, 1.0)
        invcnt[c] = (1.0 / np.maximum(cnt, 1.0)).reshape(GB, P).T.astype(np.float32)

    # edges + self-loops grouped by dst block
    e_src_g = newid[src]
    e_dst_core = core_all[dst]
    e_dst_loc = loc_all[dst]
    counts = np.zeros((N_CORES, NBLK), dtype=np.int64)
    np.add.at(counts, (e_dst_core, e_dst_loc // P), 1)
    np.add.at(counts, (core_all, loc_all // P), 1)     # self-loops
    C_b = np.maximum(np.ceil(counts.max(axis=0) / P).astype(np.int64), 1)
    col_base = np.concatenate([[0], np.cumsum(C_b)])
    NCH = int(col_base[-1])

    srcg = np.zeros((N_CORES, P, NCH), dtype=np.int32)
    dstl = np.full((N_CORES, P, NCH), -1.0, dtype=np.float32)
    for c in range(N_CORES):
        m = e_dst_core == c
        es = np.concatenate([e_src_g[m], newid[core_all == c]])
        ed = np.concatenate([e_dst_loc[m], loc_all[core_all == c]])
        order = np.argsort(ed // P, kind="stable")
        es, ed = es[order], ed[order]
        blk = ed // P
        blk_starts = np.searchsorted(blk, np.arange(NBLK))
        slot = np.arange(len(es)) - blk_starts[blk]
        col = col_base[blk] + slot // P
        row = slot % P
        srcg[c, row, col] = es.astype(np.int32)
        dstl[c, row, col] = (ed % P).astype(np.float32)

    return dict(C2=C2, NBLK=NBLK, NPC=NPC, NCH=NCH,
                C_b=tuple(int(v) for v in C_b),
                x_loc=x_loc, dinvb=dinvb, glocb=glocb, invcnt=invcnt,
                srcg=srcg, dstl=dstl)


def _build(C2, NBLK, NPC, C_b, NCH, hb2_val):
    col_base = [0]
    for v in C_b:
        col_base.append(col_base[-1] + v)
    table_dt = F16
    nc = bacc.Bacc("TRN2", target_bir_lowering=False, debug=False,
                   num_devices=N_CORES)
    x_d = nc.dram_tensor("x_loc", [NPC, H], F32, kind="ExternalInput")
    srcg_d = nc.dram_tensor("srcg", [P, NCH], I32, kind="ExternalInput")
    dstl_d = nc.dram_tensor("dstl", [P, NCH], table_dt, kind="ExternalInput")
    dinvb_d = nc.dram_tensor("dinvb", [P, NBLK], F32, kind="ExternalInput")
    glocb_d = nc.dram_tensor("glocb", [P, NBLK], F32, kind="ExternalInput")
    invcnt_d = nc.dram_tensor("invcnt", [P, GB], F32, kind="ExternalInput")
    W_d = nc.dram_tensor("Wsb", [H, 4 * H], F32, kind="ExternalInput")
    brep_d = nc.dram_tensor("brep", [P, 4 * H], F32, kind="ExternalInput")
    srep_d = nc.dram_tensor("srep", [P, 4 * H], F32, kind="ExternalInput")
    b2rep_d = nc.dram_tensor("b2rep", [P, 4 * H], F32, kind="ExternalInput")
    iota16_d = nc.dram_tensor("iota16", [P, P], table_dt, kind="ExternalInput")
    iota32_d = nc.dram_tensor("iota32", [P, P], F32, kind="ExternalInput")
    hW1_d = nc.dram_tensor("hW1", [H, H], F32, kind="ExternalInput")
    hb1rep_d = nc.dram_tensor("hb1rep", [P, H], F32, kind="ExternalInput")
    hW2_d = nc.dram_tensor("hW2", [H, 1], F32, kind="ExternalInput")
    out_d = nc.dram_tensor("out", [GPC, 1], F32, kind="ExternalOutput")

    t_loc = [nc.dram_tensor(f"t_loc{l}", [NPC, H], table_dt) for l in range(4)]
    T_full = [nc.dram_tensor(f"T_full{l}", [N_CORES * NPC, H], table_dt)
              for l in range(4)]

    with tile.TileContext(nc) as tc:
        with (
            tc.tile_pool(name="persist", bufs=1) as pp,
            tc.tile_pool(name="stagea", bufs=3) as sap,
            tc.tile_pool(name="pool2", bufs=2) as wp2,
            tc.tile_pool(name="psum_agg", bufs=1, space="PSUM") as psagg_tp,
            tc.tile_pool(name="psum_a", bufs=1, space="PSUM") as psa_tp,
            tc.tile_pool(name="psum_p", bufs=1, space="PSUM") as psp_tp,
        ):
            h_sb = pp.tile([P, NBLK * H], F32)
            t_sb = pp.tile([P, NBLK * H], table_dt)
            srcg = pp.tile([P, NCH], I32)
            dstl = pp.tile([P, NCH], table_dt)
            dinvb = pp.tile([P, NBLK], F32)
            glocb = pp.tile([P, NBLK], F32)
            invcnt = pp.tile([P, GB], F32)
            W_sb = pp.tile([H, 4 * H], F32)
            brep = pp.tile([P, 4 * H], F32)
            srep = pp.tile([P, 4 * H], F32)
            b2rep = pp.tile([P, 4 * H], F32)
            iota16 = pp.tile([P, P], table_dt)
            iota32 = pp.tile([P, P], F32)
            hW1_sb = pp.tile([H, H], F32)
            hb1rep = pp.tile([P, H], F32)
            hW2_sb = pp.tile([H, 1], F32)
            ident = pp.tile([P, P], F32)
            z2all = pp.tile([1, GPC], F32)
            for sb, d in [(srcg, srcg_d), (dstl, dstl_d), (dinvb, dinvb_d),
                          (glocb, glocb_d), (invcnt, invcnt_d), (W_sb, W_d),
                          (brep, brep_d), (srep, srep_d), (b2rep, b2rep_d),
                          (iota16, iota16_d), (iota32, iota32_d),
                          (hW1_sb, hW1_d), (hb1rep, hb1rep_d), (hW2_sb, hW2_d)]:
                nc.sync.dma_start(sb[:], d[:])
            make_identity(nc, ident[:])
            for b in range(NBLK):
                nc.sync.dma_start(h_sb[:, b * H:(b + 1) * H],
                                  x_d[b * P:(b + 1) * P, :])

            # chunk buffer sets: [half][k]
            sets = [[(pp.tile([P, H], table_dt, name=f"g{h}_{k}"),
                      pp.tile([P, P], table_dt, name=f"oh{h}_{k}"))
                     for k in range(UNROLL)] for h in range(2)]
            ps_half = [psagg_tp.tile([P, H], F32, space="PSUM", name=f"psagg{h}")
                       for h in range(2)]
            def agg_chunk(col, sk, ps, start, stop, T_l):
                g, oh = sk
                nc.gpsimd.indirect_dma_start(
                    out=g[:], out_offset=None, in_=T_l[:],
                    in_offset=bass.IndirectOffsetOnAxis(ap=srcg[:, col:col + 1],
                                                        axis=0))
                nc.vector.tensor_tensor(
                    out=oh[:], in0=dstl[:, col:col + 1].to_broadcast([P, P]),
                    in1=iota16[:], op=mybir.AluOpType.is_equal)
                nc.tensor.matmul(ps[:], lhsT=oh[:], rhs=g[:],
                                 start=start, stop=stop, skip_group_check=True)

            def emit_t_block(l, b):
                # tt_l[block b] = dinv * (h[block b] @ W_l), into t_loc[l]
                ls_t = slice(l * H, (l + 1) * H)
                trp = psa_tp.tile([P, H], F32, space="PSUM", name="trp")
                nc.tensor.transpose(out=trp[:],
                                    in_=h_sb[:, b * H:(b + 1) * H],
                                    identity=ident[:])
                hT = sap.tile([P, H], F32, name="hT")
                nc.scalar.copy(hT[:], trp[:])
                tps = psa_tp.tile([P, H], F32, space="PSUM", name="tps")
                nc.tensor.matmul(tps[:], lhsT=hT[:], rhs=W_sb[:, ls_t],
                                 start=True, stop=True, skip_group_check=True)
                nc.scalar.activation(t_sb[:, b * H:(b + 1) * H], tps[:],
                                     mybir.ActivationFunctionType.Copy,
                                     scale=dinvb[:, b:b + 1])
                nc.sync.dma_start(t_loc[l][b * P:(b + 1) * P, :],
                                  t_sb[:, b * H:(b + 1) * H])

            with nc.named_scope("stageA0"):
                for b in range(NBLK):
                    emit_t_block(0, b)

            for l in range(4):
                ls = slice(l * H, (l + 1) * H)
                with nc.named_scope(f"ag{l}"):
                    nc.gpsimd.collective_compute(
                        "AllGather", mybir.AluOpType.bypass,
                        replica_groups=[list(range(N_CORES))],
                        ins=[t_loc[l][:]], outs=[T_full[l][:]])

                with nc.named_scope(f"agg{l}"):
                    for bp in range(NBLK // 2):
                        blocks = [2 * bp, 2 * bp + 1]
                        cb = [C_b[blocks[0]], C_b[blocks[1]]]
                        for j in range(max(cb)):
                            for half in range(2):
                                if j < cb[half]:
                                    agg_chunk(col_base[blocks[half]] + j,
                                              sets[half][j % UNROLL],
                                              ps_half[half], j == 0,
                                              j == cb[half] - 1, T_full[l])
                        for half in range(2):
                            # epilogue: h = BN(relu(dinv*psum + b))
                            b = blocks[half]
                            ps = ps_half[half]
                            e0 = wp2.tile([P, H], F32, name=f"e0_{half}")
                            e1 = wp2.tile([P, H], F32, name=f"e1_{half}")
                            nc.vector.tensor_scalar(
                                e0[:], ps[:], dinvb[:, b:b + 1], None,
                                mybir.AluOpType.mult)
                            nc.vector.tensor_tensor(
                                out=e1[:], in0=e0[:], in1=brep[:, ls],
                                op=mybir.AluOpType.add)
                            nc.scalar.activation(
                                e0[:], e1[:], mybir.ActivationFunctionType.Relu)
                            nc.vector.tensor_tensor(
                                out=e1[:], in0=e0[:], in1=srep[:, ls],
                                op=mybir.AluOpType.mult)
                            nc.vector.tensor_tensor(
                                out=h_sb[:, b * H:(b + 1) * H], in0=e1[:],
                                in1=b2rep[:, ls],
                                op=mybir.AluOpType.add)
                            if l < 3:
                                emit_t_block(l + 1, b)

            # ---- global mean pool + head
            with nc.named_scope("pool"):
                for gb in range(GB):
                    pps = psp_tp.tile([P, H], F32, space="PSUM", name="pps")
                    for k in range(C2):
                        b = gb * C2 + k
                        oh32 = wp2.tile([P, P], F32, name="oh32")
                        nc.vector.tensor_tensor(
                            out=oh32[:],
                            in0=glocb[:, b:b + 1].to_broadcast([P, P]),
                            in1=iota32[:], op=mybir.AluOpType.is_equal)
                        nc.tensor.matmul(pps[:], lhsT=oh32[:],
                                         rhs=h_sb[:, b * H:(b + 1) * H],
                                         start=(k == 0), stop=(k == C2 - 1),
                                         skip_group_check=True)
                    pooled = wp2.tile([P, H], F32, name="pooled")
                    nc.vector.tensor_scalar(pooled[:], pps[:],
                                            invcnt[:, gb:gb + 1], None,
                                            mybir.AluOpType.mult)
                    # head: relu(pooled @ hW1 + hb1) @ hW2 + hb2
                    trp = psp_tp.tile([P, H], F32, space="PSUM", name="htr")
                    nc.tensor.transpose(out=trp[:], in_=pooled[:],
                                        identity=ident[:])
                    poolT = wp2.tile([P, H], F32, name="poolT")
                    nc.scalar.copy(poolT[:], trp[:])
                    z1ps = psp_tp.tile([P, H], F32, space="PSUM", name="z1ps")
                    nc.tensor.matmul(z1ps[:], lhsT=poolT[:], rhs=hW1_sb[:],
                                     start=True, stop=True,
                                     skip_group_check=True)
                    r1 = wp2.tile([P, H], F32, name="r1")
                    nc.vector.tensor_tensor(out=r1[:], in0=z1ps[:],
                                            in1=hb1rep[:],
                                            op=mybir.AluOpType.add)
                    nc.scalar.activation(r1[:], r1[:],
                                         mybir.ActivationFunctionType.Relu)
                    tr2 = psp_tp.tile([P, H], F32, space="PSUM", name="htr")
                    nc.tensor.transpose(out=tr2[:], in_=r1[:], identity=ident[:])
                    r1T = wp2.tile([P, H], F32, name="r1T")
                    nc.scalar.copy(r1T[:], tr2[:])
                    z2ps = psp_tp.tile([1, P], F32, space="PSUM", name="z2ps")
                    nc.tensor.matmul(z2ps[:], lhsT=hW2_sb[:], rhs=r1T[:],
                                     start=True, stop=True,
                                     skip_group_check=True)
                    nc.vector.tensor_scalar(
                        z2all[0:1, gb * P:(gb + 1) * P], z2ps[:],
                        float(hb2_val), None, mybir.AluOpType.add)
                nc.sync.dma_start(out_d[:, 0:1], z2all[0:1, :])

    nc.compile()
    return nc


def kernel(**inputs):
    global LAST_EXEC_NS
    x = np.ascontiguousarray(np.asarray(inputs["x"], dtype=np.float32))
    ei = np.asarray(inputs["edge_index"]).astype(np.int64)
    batch = np.asarray(inputs["batch"]).astype(np.int64)
    Ws = np.asarray(inputs["Ws"], dtype=np.float32)
    bs = np.asarray(inputs["bs"], dtype=np.float32)
    gammas = np.asarray(inputs["gammas"], dtype=np.float32)
    betas = np.asarray(inputs["betas"], dtype=np.float32)
    bn_means = np.asarray(inputs["bn_means"], dtype=np.float32)
    bn_vars = np.asarray(inputs["bn_vars"], dtype=np.float32)
    hW1 = np.asarray(inputs["hW1"], dtype=np.float32)
    hb1 = np.asarray(inputs["hb1"], dtype=np.float32)
    hW2 = np.asarray(inputs["hW2"], dtype=np.float32)
    hb2 = np.asarray(inputs["hb2"], dtype=np.float32)

    src, dst = ei[0], ei[1]
    N = x.shape[0]
    deg = np.bincount(dst, minlength=N).astype(np.float64) + 1.0
    dinv = (1.0 / np.sqrt(deg)).astype(np.float32)

    meta = _preprocess(x, src, dst, batch, dinv)
    C2, NBLK, NPC, C_b, NCH = (meta[k] for k in
                               ("C2", "NBLK", "NPC", "C_b", "NCH"))

    key = (C2, NBLK, NPC, C_b, NCH, float(hb2[0]))
    if key not in _CACHE:
        _CACHE[key] = _build(C2, NBLK, NPC, C_b, NCH, float(hb2[0]))
    nc = _CACHE[key]

    # replicated constant arrays
    s_l = gammas / np.sqrt(bn_vars + BN_EPS)            # [4, H]
    b2_l = betas - bn_means * s_l                        # [4, H]
    Wsb = np.ascontiguousarray(Ws.transpose(1, 0, 2).reshape(H, 4 * H))
    brep = np.broadcast_to(bs.reshape(1, 4 * H), (P, 4 * H)).copy()
    srep = np.broadcast_to(s_l.reshape(1, 4 * H), (P, 4 * H)).copy()
    b2rep = np.broadcast_to(b2_l.reshape(1, 4 * H), (P, 4 * H)).copy()
    iota16 = np.broadcast_to(np.arange(P, dtype=np.float16)[None, :],
                             (P, P)).copy()
    iota32 = iota16.astype(np.float32)
    hb1rep = np.broadcast_to(hb1[None, :], (P, H)).copy()

    in_maps = []
    for c in range(N_CORES):
        in_maps.append({
            "x_loc": meta["x_loc"][c],
            "srcg": meta["srcg"][c],
            "dstl": meta["dstl"][c].astype(np.float16),
            "dinvb": meta["dinvb"][c],
            "glocb": meta["glocb"][c],
            "invcnt": meta["invcnt"][c],
            "Wsb": Wsb, "brep": brep, "srep": srep, "b2rep": b2rep,
            "iota16": iota16, "iota32": iota32,
            "hW1": hW1, "hb1rep": hb1rep, "hW2": hW2,
        })

    trace = os.environ.get("BASS_GCN_TRACE", "") == "1"
    if trace:
        bass_utils.upload_artifacts = lambda tmpdir: "local://" + tmpdir
    res = bass_utils.run_bass_kernel_spmd(nc, in_maps, list(range(N_CORES)),
                                          trace=trace)
    LAST_EXEC_NS = res.exec_time_ns
    if res.exec_time_ns is not None:
        print(f"HW exec time: {res.exec_time_ns} ns")

    out = np.concatenate([res.results[c]["out"] for c in range(N_CORES)],
                         axis=0).astype(np.float32)
    return out
